# revision 4
# baseline (speedup 1.0000x reference)
"""DualRoadGNN Trainium2 kernel: 8-core SPMD, sharded by graph.

Layout: feature-major per graph ([H partitions, node columns]); graphs padded
500 -> 512 nodes. GCN message passing runs as dense matmuls against per-graph
adjacency matrices built on device from host-shipped integer edge lists
(local_scatter of multiplicities, scaled by rsqrt-degree outer products).
KNN road: cosine sim via PE matmul, top-k via DVE max/max_index, adjacency
via local_scatter + PE transpose.
"""
import contextlib
import os
import sys

sys.path.insert(0, "/opt/trn_rl_repo")
import numpy as np

import concourse.bacc as bacc
import concourse.tile as tile
from concourse import mybir
from concourse.bass_utils import run_bass_kernel_spmd
from concourse.masks import make_identity

G, NPG, NP = 100, 500, 512
IN, H, L = 128, 256, 2   # L = executed layer iterations (range(3-1) in the model)
W = 64                   # max unique out-edges per source node (incl self loop)
N_CORES = 8
GPC = 13                 # graph slots per core
STARTS = [0, 13, 26, 39, 52, 64, 76, 88, 100]
NGS = [STARTS[i + 1] - STARTS[i] for i in range(N_CORES)]
F32 = mybir.dt.float32
BF16 = mybir.dt.bfloat16
MM_DT = mybir.dt.float32r if os.environ.get("KERNEL_F32R", "1") == "1" else F32

# fvec column map
FV_EMB_B = 0
FV_GATE_B = 2
FV_L = 4   # then per layer: conv_b, norm_w, norm_b, norm_ms, fconv_b, fnorm_w, fnorm_b, fnorm_ms
FV_N = 4 + L * 16


def _mm(ap):
    return ap.bitcast(MM_DT) if MM_DT is not F32 else ap


def build_program(gpc):
    nc = bacc.Bacc("TRN2", target_bir_lowering=False, debug=False, num_devices=N_CORES)
    d = {}
    d["xT"] = nc.dram_tensor("xT", [gpc, IN, NP], F32, kind="ExternalInput")
    d["ei"] = nc.dram_tensor("ei", [gpc, 4, 128, W], mybir.dt.int16, kind="ExternalInput")
    d["ev"] = nc.dram_tensor("ev", [gpc, 4, 128, W], F32, kind="ExternalInput")
    d["degpc"] = nc.dram_tensor("degpc", [128, gpc * 4], F32, kind="ExternalInput")
    d["degrow"] = nc.dram_tensor("degrow", [gpc, NP], F32, kind="ExternalInput")
    d["embW"] = nc.dram_tensor("embW", [IN, H], F32, kind="ExternalInput")
    d["convW"] = nc.dram_tensor("convW", [L, H, H], F32, kind="ExternalInput")
    d["fconvW"] = nc.dram_tensor("fconvW", [L, H, H], F32, kind="ExternalInput")
    d["gateW"] = nc.dram_tensor("gateW", [2 * H, H], F32, kind="ExternalInput")
    d["fvec"] = nc.dram_tensor("fvec", [128, FV_N], F32, kind="ExternalInput")
    d["gf"] = nc.dram_tensor("gf", [gpc, H], F32, kind="ExternalOutput")

    with tile.TileContext(nc) as tc:
        _emit(nc, tc, gpc, d)
    nc.compile()
    return nc


def _emit(nc, tc, gpc, d):
    AF = mybir.ActivationFunctionType
    OP = mybir.AluOpType
    X = mybir.AxisListType.X

    ctx = contextlib.ExitStack()
    with ctx:
        sg = ctx.enter_context(tc.tile_pool(name="singles", bufs=1))
        pg = ctx.enter_context(tc.tile_pool(name="pg", bufs=2))
        psA = ctx.enter_context(tc.tile_pool(name="psA", bufs=3, space="PSUM"))
        psM = ctx.enter_context(tc.tile_pool(name="psM", bufs=2, space="PSUM"))
        psT = ctx.enter_context(tc.tile_pool(name="psT", bufs=2, space="PSUM"))
        psN = ctx.enter_context(tc.tile_pool(name="psN", bufs=1, space="PSUM"))

        def T(shape, dtype=F32, tag=None, pool=pg):
            return pool.tile(shape, dtype, name=tag, tag=tag)

        # --- resident constants/weights ---
        embW = T([128, H], tag="embW_t", pool=sg)
        nc.sync.dma_start(out=embW, in_=d["embW"][:, :])
        convW = {}
        for l in range(L):
            for k in range(2):
                t = T([128, H], tag=f"convW{l}_{k}", pool=sg)
                nc.sync.dma_start(out=t, in_=d["convW"][l, k * 128:(k + 1) * 128, :])
                convW[(l, k)] = t
                t2 = T([128, H], tag=f"fconvW{l}_{k}", pool=sg)
                nc.sync.dma_start(out=t2, in_=d["fconvW"][l, k * 128:(k + 1) * 128, :])
                convW[(l, k, "f")] = t2
        gateW = []
        for c in range(4):
            t = T([128, H], tag=f"gateW{c}", pool=sg)
            nc.sync.dma_start(out=t, in_=d["gateW"][c * 128:(c + 1) * 128, :])
            gateW.append(t)
        fvec = T([128, FV_N], tag="fvec_t", pool=sg)
        nc.sync.dma_start(out=fvec, in_=d["fvec"][:, :])

        degpc = T([128, gpc * 4], tag="degpc_t", pool=sg)
        nc.sync.dma_start(out=degpc, in_=d["degpc"][:, :])
        dinvpc = T([128, gpc * 4], tag="dinvpc", pool=sg)
        nc.vector.reciprocal(out=dinvpc, in_=degpc)
        nc.scalar.sqrt(dinvpc, dinvpc)

        ident = T([128, 128], tag="ident", pool=sg)
        make_identity(nc, ident)
        diagq = T([128, 128], tag="diagq", pool=sg)
        nc.scalar.mul(diagq, ident, 0.25)
        ones128 = T([128, 1], tag="ones128", pool=sg)
        nc.vector.memset(ones128, 1.0)
        ones1 = T([1, 128], tag="ones1", pool=sg)
        nc.vector.memset(ones1, 1.0)
        epsT = T([128, 1], tag="epsT", pool=sg)
        nc.vector.memset(epsT, 1e-5)
        q4 = T([128, 4], BF16, tag="q4", pool=sg)
        nc.vector.memset(q4, 0.25)

        def fv(col):
            return fvec[:, col:col + 1]

        def road(inT, Wk0, Wk1, Amat, b_col, nw_col, nb_col, nms_col, otag):
            # m = h @ W (node-major, per source chunk), then out^T = m^T-free matmul vs A^T
            m = []
            for sc in range(4):
                ps = psM.tile([128, H], F32, name="psm", tag="psm", bufs=2)
                nc.tensor.matmul(ps, lhsT=_mm(inT[0][:, sc * 128:(sc + 1) * 128]), rhs=_mm(Wk0), start=True, stop=False)
                nc.tensor.matmul(ps, lhsT=_mm(inT[1][:, sc * 128:(sc + 1) * 128]), rhs=_mm(Wk1), start=False, stop=True)
                mt = T([128, H], tag=f"m_{sc}")
                nc.scalar.copy(mt, ps)
                m.append(mt)
            outT = []
            for k in range(2):
                ps = psA.tile([128, NP], F32, name="psbig", tag="psbig", bufs=3)
                for sc in range(4):
                    nc.tensor.matmul(ps, lhsT=_mm(m[sc][:, k * 128:(k + 1) * 128]), rhs=_mm(Amat[sc]),
                                     start=(sc == 0), stop=(sc == 3))
                cT = T([128, NP], tag="cT")
                nc.scalar.activation(out=cT, in_=ps, func=AF.Identity, bias=fv(b_col + k))
                # GraphNorm (+ leaky relu folded into the final activation)
                s1 = T([128, 1], tag="s1")
                nc.vector.reduce_sum(out=s1, in_=cT[:, 0:NPG], axis=X)
                msn = T([128, 1], tag="msn")
                nc.vector.tensor_scalar(out=msn, in0=s1, scalar1=fv(nms_col + k), scalar2=-1.0 / NPG,
                                        op0=OP.mult, op1=OP.mult)
                junk = T([128, NPG], tag="junk")
                ssq = T([128, 1], tag="ssq")
                nc.scalar.activation(out=junk, in_=cT[:, 0:NPG], func=AF.Square, bias=msn[:, 0:1],
                                     accum_out=ssq[:, 0:1])
                std = T([128, 1], tag="std")
                nc.scalar.activation(out=std, in_=ssq, func=AF.Sqrt, bias=epsT[:, 0:1], scale=1.0 / NPG)
                rstd = T([128, 1], tag="rstd")
                nc.vector.reciprocal(out=rstd, in_=std)
                wr = T([128, 1], tag="wr")
                nc.vector.tensor_single_scalar(out=wr, in_=rstd, scalar=fv(nw_col + k), op=OP.mult)
                b2 = T([128, 1], tag="b2")
                nc.vector.tensor_scalar(out=b2, in0=wr, scalar1=msn[:, 0:1], scalar2=fv(nb_col + k),
                                        op0=OP.mult, op1=OP.add)
                oT = T([128, NP], tag=f"{otag}_{k}")
                nc.scalar.activation(out=oT, in_=cT, func=AF.Lrelu, bias=b2[:, 0:1], scale=wr[:, 0:1], alpha=0.01)
                outT.append(oT)
            return outT

        for i in range(gpc):
            # ---- loads ----
            xT = T([128, NP], tag="xT_t")
            nc.sync.dma_start(out=xT, in_=d["xT"][i])
            eit = T([128, 4, W], mybir.dt.int16, tag="eit")
            evf = T([128, 4, W], tag="evf")
            for c in range(4):
                nc.sync.dma_start(out=eit[:, c, :], in_=d["ei"][i, c])
                nc.sync.dma_start(out=evf[:, c, :], in_=d["ev"][i, c])
            evb = T([128, 4, W], BF16, tag="evb")
            nc.vector.tensor_copy(out=evb, in_=evf)
            drow = T([1, NP], tag="drow")
            nc.sync.dma_start(out=drow, in_=d["degrow"][i:i + 1, :])

            # ---- dinv broadcast row ----
            nc.vector.reciprocal(out=drow, in_=drow)
            nc.scalar.sqrt(drow, drow)
            psB = psA.tile([128, NP], F32, name="psbig", tag="psbig", bufs=3)
            nc.tensor.matmul(psB, lhsT=ones1[:, :], rhs=drow, start=True, stop=True)
            dinvB = T([128, NP], tag="dinvB")
            nc.vector.tensor_copy(out=dinvB, in_=psB)

            # ---- embedding ----
            hT = []
            for k in range(2):
                ps = psA.tile([128, NP], F32, name="psbig", tag="psbig", bufs=3)
                nc.tensor.matmul(ps, lhsT=_mm(embW[:, k * 128:(k + 1) * 128]), rhs=_mm(xT), start=True, stop=True)
                t = T([128, NP], tag=f"hT_{k}")
                nc.scalar.activation(out=t, in_=ps, func=AF.Identity, bias=fv(FV_EMB_B + k))
                hT.append(t)

            # ---- main-road adjacency AT[s-chunk][s, d] ----
            AT = []
            for c in range(4):
                am = T([128, NP], BF16, tag="am")
                nc.gpsimd.local_scatter(out_ap=am[:, :], data_ap=evb[:, c, :], idxs_ap=eit[:, c, :],
                                        channels=128, num_elems=NP, num_idxs=W)
                amf = T([128, NP], tag="amf")
                nc.vector.tensor_copy(out=amf, in_=am)
                at = T([128, NP], tag=f"AT_{c}")
                nc.vector.scalar_tensor_tensor(out=at, in0=amf, scalar=dinvpc[:, i * 4 + c:i * 4 + c + 1],
                                               in1=dinvB, op0=OP.mult, op1=OP.mult)
                AT.append(at)

            # ---- knn road: cosine sim + top-k -> AfT ----
            sq = []
            for k in range(2):
                t = T([128, NP], tag=f"sq_{k}")
                nc.scalar.square(t, hT[k])
                sq.append(t)
            psn = psN.tile([1, NP], F32, name="psn", tag="psn", bufs=1)
            nc.tensor.matmul(psn, lhsT=_mm(ones128[:, :]), rhs=_mm(sq[0]), start=True, stop=False)
            nc.tensor.matmul(psn, lhsT=_mm(ones128[:, :]), rhs=_mm(sq[1]), start=False, stop=True)
            rin = T([1, NP], tag="rin")
            nc.scalar.sqrt(rin, psn)
            nc.vector.tensor_scalar_add(rin, rin, 1e-12)
            nc.vector.reciprocal(out=rin, in_=rin)
            psR = psA.tile([128, NP], F32, name="psbig", tag="psbig", bufs=3)
            nc.tensor.matmul(psR, lhsT=ones1[:, :], rhs=rin, start=True, stop=True)
            rb = T([128, NP], tag="rb")
            nc.vector.tensor_copy(out=rb, in_=psR)
            hnT = []
            for k in range(2):
                t = T([128, NP], tag=f"hnT_{k}")
                nc.vector.tensor_mul(t, hT[k], rb)
                hnT.append(t)

            AfT = [T([128, NP], tag=f"AfT_{s}") for s in range(4)]
            for j in range(4):
                ps = psA.tile([128, NP], F32, name="psbig", tag="psbig", bufs=3)
                nc.tensor.matmul(ps, lhsT=_mm(hnT[0][:, j * 128:(j + 1) * 128]), rhs=_mm(hnT[0]), start=True, stop=False)
                nc.tensor.matmul(ps, lhsT=_mm(hnT[1][:, j * 128:(j + 1) * 128]), rhs=_mm(hnT[1]), start=False, stop=True)
                sim = T([128, NP], tag="sim")
                nc.vector.tensor_copy(out=sim, in_=ps)
                nc.vector.memset(sim[:, NPG:NP], -1e30)
                mx = T([128, 8], tag="mx")
                mi = T([128, 8], mybir.dt.uint16, tag="mi")
                nc.vector.max(mx, sim)
                nc.vector.max_index(mi, mx, sim)
                idx4 = T([128, 4], mybir.dt.int16, tag="idx4")
                nc.vector.memset(idx4, -1)
                rows = NPG - 384 if j == 3 else 128
                nc.vector.tensor_copy(out=idx4[0:rows, 0:3], in_=mi[0:rows, 0:3].bitcast(mybir.dt.int16))
                af = T([128, NP], BF16, tag="afb")
                nc.gpsimd.local_scatter(out_ap=af[:, :], data_ap=q4[:, :], idxs_ap=idx4[:, :],
                                        channels=128, num_elems=NP, num_idxs=4)
                aff = T([128, NP], tag="aff")
                nc.vector.tensor_copy(out=aff, in_=af)
                for s in range(4):
                    pst = psT.tile([128, 128], F32, name="pst", tag="pst", bufs=2)
                    nc.tensor.transpose(out=pst, in_=aff[:, s * 128:(s + 1) * 128], identity=ident)
                    nc.vector.tensor_copy(out=AfT[s][:, j * 128:(j + 1) * 128], in_=pst)
            for s in range(4):
                if s < 3:
                    nc.vector.tensor_add(AfT[s][:, s * 128:(s + 1) * 128], AfT[s][:, s * 128:(s + 1) * 128], diagq)
                else:
                    nc.vector.tensor_add(AfT[3][0:NPG - 384, 384:NPG], AfT[3][0:NPG - 384, 384:NPG],
                                         diagq[0:NPG - 384, 0:NPG - 384])

            # ---- layers ----
            prevT = hT
            curT = hT
            all0 = None
            for l in range(L):
                base = FV_L + l * 16
                h1 = road(curT, convW[(l, 0)], convW[(l, 1)], AT, base + 0, base + 2, base + 4, base + 6, "h1")
                h2 = road(h1, convW[(l, 0, "f")], convW[(l, 1, "f")], AfT, base + 8, base + 10, base + 12, base + 14, "h2")
                newT = []
                for k in range(2):
                    ps = psA.tile([128, NP], F32, name="psbig", tag="psbig", bufs=3)
                    for c in range(4):
                        rhs = h1[c] if c < 2 else h2[c - 2]
                        nc.tensor.matmul(ps, lhsT=_mm(gateW[c][:, k * 128:(k + 1) * 128]), rhs=_mm(rhs),
                                         start=(c == 0), stop=(c == 3))
                    gT = T([128, NP], tag="gT")
                    nc.scalar.activation(out=gT, in_=ps, func=AF.Sigmoid, bias=fv(FV_GATE_B + k))
                    dT = T([128, NP], tag="dT")
                    nc.vector.tensor_sub(dT, h1[k], h2[k])
                    t2 = T([128, NP], tag="t2")
                    nc.vector.tensor_mul(t2, gT, dT)
                    nc.vector.tensor_add(t2, t2, h2[k])
                    hn = T([128, NP], tag=f"hn{l}_{k}")
                    nc.vector.tensor_add(hn, t2, prevT[k])
                    newT.append(hn)
                if l == 0:
                    all0 = newT
                prevT = newT
                curT = newT

            # ---- pooling: gf = (pool(all0) + 2*pool(all1)) / NPG ----
            gfo = T([128, 2], tag="gfo")
            for k in range(2):
                r0 = T([128, 1], tag="r0")
                nc.vector.reduce_sum(out=r0, in_=all0[k][:, 0:NPG], axis=X)
                r1 = T([128, 1], tag="r1")
                nc.vector.reduce_sum(out=r1, in_=curT[k][:, 0:NPG], axis=X)
                nc.vector.scalar_tensor_tensor(out=gfo[:, k:k + 1], in0=r1, scalar=2.0, in1=r0,
                                               op0=OP.mult, op1=OP.add)
            nc.vector.tensor_scalar_mul(gfo, gfo, 1.0 / NPG)
            nc.sync.dma_start(out=d["gf"][i].rearrange("(k p) -> p k", p=128), in_=gfo)


def prep_inputs(inputs):
    """Build the 8 per-core input maps from full-problem inputs."""
    x = np.asarray(inputs["x"], np.float32)
    edge_index = np.asarray(inputs["edge_index"], np.int64)
    batch = np.asarray(inputs["batch"], np.int64)
    N = G * NPG
    assert x.shape == (N, IN)
    assert np.array_equal(batch, np.repeat(np.arange(G), NPG)), "non-uniform batch unsupported"

    src, dst = edge_index[0], edge_index[1]
    gs = src // NPG
    assert np.array_equal(dst // NPG, gs), "cross-graph edges unsupported"
    sl = src % NPG
    dl = dst % NPG

    deg = np.bincount(dst, minlength=N).astype(np.float32) + 1.0

    # unique (g, s, d) with multiplicity, self loops appended
    gg = np.arange(G, dtype=np.int64).repeat(NPG)
    nn = np.tile(np.arange(NPG, dtype=np.int64), G)
    g_all = np.concatenate([gs, gg])
    s_all = np.concatenate([sl, nn])
    d_all = np.concatenate([dl, nn])
    key = (g_all * NPG + s_all) * NPG + d_all
    uk, cnt = np.unique(key, return_counts=True)
    ud = (uk % NPG).astype(np.int16)
    row = (uk // NPG).astype(np.int64)  # g*NPG + s
    row_start = np.searchsorted(row, np.arange(N))
    pos = np.arange(len(row)) - row_start[row]
    assert pos.max() < W, f"out-degree overflow: {pos.max() + 1} > {W}"
    EI = np.full((N, W), -1, np.int16)
    EV = np.zeros((N, W), np.float32)
    EI[row, pos] = ud
    EV[row, pos] = cnt

    rng = np.random.default_rng(12345)
    wts = dict(
        embW=np.ascontiguousarray(np.asarray(inputs["emb_W"], np.float32)),
        convW=np.ascontiguousarray(np.asarray(inputs["conv_W"], np.float32)[:L]),
        fconvW=np.ascontiguousarray(np.asarray(inputs["fconv_W"], np.float32)[:L]),
        gateW=np.ascontiguousarray(np.asarray(inputs["gate_W"], np.float32)),
    )
    fvec = np.zeros((128, FV_N), np.float32)

    def setv(col, vec):
        fvec[:, col] = vec[0:128]
        fvec[:, col + 1] = vec[128:256]

    setv(FV_EMB_B, np.asarray(inputs["emb_b"], np.float32))
    setv(FV_GATE_B, np.asarray(inputs["gate_b"], np.float32))
    for l in range(L):
        base = FV_L + l * 16
        setv(base + 0, np.asarray(inputs["conv_b"], np.float32)[l])
        setv(base + 2, np.asarray(inputs["norm_w"], np.float32)[l])
        setv(base + 4, np.asarray(inputs["norm_b"], np.float32)[l])
        setv(base + 6, np.asarray(inputs["norm_ms"], np.float32)[l])
        setv(base + 8, np.asarray(inputs["fconv_b"], np.float32)[l])
        setv(base + 10, np.asarray(inputs["fnorm_w"], np.float32)[l])
        setv(base + 12, np.asarray(inputs["fnorm_b"], np.float32)[l])
        setv(base + 14, np.asarray(inputs["fnorm_ms"], np.float32)[l])

    in_maps = []
    for c in range(N_CORES):
        g0, ng = STARTS[c], NGS[c]
        xT = np.zeros((GPC, IN, NP), np.float32)
        ei_c = np.full((GPC, 4, 128, W), -1, np.int16)
        ev_c = np.zeros((GPC, 4, 128, W), np.float32)
        degpc = np.ones((GPC, 4, 128), np.float32)
        degrow = np.ones((GPC, NP), np.float32)
        for j in range(GPC):
            if j < ng:
                g = g0 + j
                xg = x[g * NPG:(g + 1) * NPG]
            else:
                xg = rng.standard_normal((NPG, IN)).astype(np.float32)
            xT[j, :, 0:NPG] = xg.T
            if j < ng:
                eig = np.full((NP, W), -1, np.int16)
                evg = np.zeros((NP, W), np.float32)
                eig[0:NPG] = EI[g * NPG:(g + 1) * NPG]
                evg[0:NPG] = EV[g * NPG:(g + 1) * NPG]
                ei_c[j] = eig.reshape(4, 128, W)
                ev_c[j] = evg.reshape(4, 128, W)
                dg = np.ones(NP, np.float32)
                dg[0:NPG] = deg[g * NPG:(g + 1) * NPG]
                degpc[j] = dg.reshape(4, 128)
                degrow[j] = dg
        in_maps.append(dict(
            xT=xT,
            ei=ei_c,
            ev=ev_c,
            degpc=np.ascontiguousarray(degpc.reshape(GPC * 4, 128).T),
            degrow=degrow,
            fvec=fvec,
            **wts,
        ))
    return in_maps


_prog_cache = {}


def _get_program():
    if "nc" not in _prog_cache:
        _prog_cache["nc"] = build_program(GPC)
    return _prog_cache["nc"]


def kernel(**inputs):
    in_maps = prep_inputs(inputs)
    nc = _get_program()
    trace = os.environ.get("KERNEL_TRACE", "0") == "1"
    kw = {}
    if trace:
        import antenv
        p = "/opt/trn_rl_repo/antenv"
        if p not in antenv.__path__:
            antenv.__path__.append(p)
        from antenv.axon_hooks import get_axon_ntff_profile_hook, set_axon_ntff_profile_hook
        if get_axon_ntff_profile_hook() is None:
            from trn_agent_boot.trn_boot import _ntff_profile_via_ctypes
            set_axon_ntff_profile_hook(_ntff_profile_via_ctypes("/opt/axon/libaxon_pjrt.so"))
        from concourse import bass_utils as _bu
        _bu.upload_artifacts = lambda tmpdir: "local://" + tmpdir
        base = os.environ.get("KERNEL_TRACE_DIR")
        if base:
            _prog_cache["run_id"] = _prog_cache.get("run_id", 0) + 1
            tdir = os.path.join(base, f"run{_prog_cache['run_id']}")
            os.makedirs(tdir, exist_ok=True)
        else:
            tdir = None
        kw = dict(trace=True, tmpdir=tdir)
    res = run_bass_kernel_spmd(nc, in_maps, core_ids=list(range(N_CORES)), **kw)
    if trace:
        print(f"HW exec time: {res.exec_time_ns} ns")
    out = np.zeros((G, H), np.float32)
    for c in range(N_CORES):
        g0, ng = STARTS[c], NGS[c]
        out[g0:g0 + ng] = res.results[c]["gf"][0:ng]
    return out


# revision 9
# speedup vs baseline: 1.3116x; 1.3116x over previous
"""DualRoadGNN Trainium2 kernel: 8-core SPMD, sharded by graph.

Layout: feature-major per graph ([H partitions, node columns]); graphs padded
500 -> 512 nodes. GCN message passing runs as dense matmuls against per-graph
adjacency matrices built on device from host-shipped integer edge lists
(local_scatter of multiplicities, scaled by rsqrt-degree outer products).
KNN road: cosine sim via PE matmul, top-k via DVE max/max_index, adjacency
via local_scatter + PE transpose.
"""
import contextlib
import os
import sys

sys.path.insert(0, "/opt/trn_rl_repo")
import numpy as np

import concourse.bacc as bacc
import concourse.tile as tile
from concourse import mybir
from concourse.bass_utils import run_bass_kernel_spmd
from concourse.masks import make_identity

G, NPG, NP = 100, 500, 512
IN, H, L = 128, 256, 2   # L = executed layer iterations (range(3-1) in the model)
W = 64                   # max unique out-edges per source node (incl self loop)
N_CORES = 8
GPC = 13                 # graph slots per core
STARTS = [0, 13, 26, 39, 52, 64, 76, 88, 100]
NGS = [STARTS[i + 1] - STARTS[i] for i in range(N_CORES)]
F32 = mybir.dt.float32
BF16 = mybir.dt.bfloat16
HDT = mybir.dt.float32r if os.environ.get("KERNEL_F32R", "1") == "1" else F32

# fvec column map
FV_EMB_B = 0
FV_GATE_B = 2
FV_L = 4   # then per layer: conv_b, norm_w, norm_b, norm_ms, fconv_b, fnorm_w, fnorm_b, fnorm_ms
FV_N = 4 + L * 16


def build_program(gpc):
    nc = bacc.Bacc("TRN2", target_bir_lowering=False, debug=False, num_devices=N_CORES)
    d = {}
    d["xT"] = nc.dram_tensor("xT", [gpc, IN, NP], HDT, kind="ExternalInput")
    d["ei"] = nc.dram_tensor("ei", [gpc, 4, 128, W], mybir.dt.int16, kind="ExternalInput")
    d["ev"] = nc.dram_tensor("ev", [gpc, 4, 128, W], F32, kind="ExternalInput")
    d["degpc"] = nc.dram_tensor("degpc", [128, gpc * 4], F32, kind="ExternalInput")
    d["degrow"] = nc.dram_tensor("degrow", [gpc, NP], F32, kind="ExternalInput")
    d["embW"] = nc.dram_tensor("embW", [IN, H], HDT, kind="ExternalInput")
    d["convW"] = nc.dram_tensor("convW", [L, H, H], HDT, kind="ExternalInput")
    d["fconvW"] = nc.dram_tensor("fconvW", [L, H, H], HDT, kind="ExternalInput")
    d["gateW"] = nc.dram_tensor("gateW", [2 * H, H], HDT, kind="ExternalInput")
    d["fvec"] = nc.dram_tensor("fvec", [128, FV_N], F32, kind="ExternalInput")
    d["gf"] = nc.dram_tensor("gf", [gpc, H], F32, kind="ExternalOutput")

    with tile.TileContext(nc) as tc:
        _emit(nc, tc, gpc, d)
    nc.compile()
    return nc


def _emit(nc, tc, gpc, d):
    AF = mybir.ActivationFunctionType
    OP = mybir.AluOpType
    X = mybir.AxisListType.X
    I32 = mybir.dt.int32

    ctx = contextlib.ExitStack()
    with ctx:
        sg = ctx.enter_context(tc.tile_pool(name="singles", bufs=1))
        pg = ctx.enter_context(tc.tile_pool(name="pg", bufs=2))
        psA = ctx.enter_context(tc.tile_pool(name="psA", bufs=3, space="PSUM"))
        psM = ctx.enter_context(tc.tile_pool(name="psM", bufs=2, space="PSUM"))
        psT = ctx.enter_context(tc.tile_pool(name="psT", bufs=2, space="PSUM"))
        psN = ctx.enter_context(tc.tile_pool(name="psN", bufs=1, space="PSUM"))

        def T(shape, dtype=F32, tag=None, pool=pg):
            return pool.tile(shape, dtype, name=tag, tag=tag)

        def f32(ap):
            # read-view of an HDT tile for non-matmul engines
            return ap.bitcast(F32) if ap.dtype != F32 else ap

        def newton_rsqrt(v_ap, out_tile, tmp_tile, iters):
            """out = 1/sqrt(v) via bit-trick + Newton (DVE only, no ACT tables).
            v_ap, out_tile, tmp_tile all same shape, f32 (v > 0)."""
            y = out_tile
            nc.vector.tensor_scalar(out=y.bitcast(I32), in0=v_ap.bitcast(I32), scalar1=1, scalar2=None,
                                    op0=OP.arith_shift_right)
            nc.vector.tensor_scalar(out=y.bitcast(I32), in0=y.bitcast(I32), scalar1=-1, scalar2=0x5F3759DF,
                                    op0=OP.mult, op1=OP.add)
            for _ in range(iters):
                nc.vector.tensor_mul(tmp_tile, y, y)
                nc.vector.tensor_mul(tmp_tile, tmp_tile, v_ap)
                nc.vector.tensor_scalar(out=tmp_tile, in0=tmp_tile, scalar1=-0.5, scalar2=1.5,
                                        op0=OP.mult, op1=OP.add)
                nc.vector.tensor_mul(y, y, tmp_tile)
            return y

        # --- resident constants/weights ---
        embW = T([128, H], HDT, tag="embW_t", pool=sg)
        nc.sync.dma_start(out=embW, in_=d["embW"][:, :])
        convW = {}
        for l in range(L):
            for k in range(2):
                t = T([128, H], HDT, tag=f"convW{l}_{k}", pool=sg)
                nc.sync.dma_start(out=t, in_=d["convW"][l, k * 128:(k + 1) * 128, :])
                convW[(l, k)] = t
                t2 = T([128, H], HDT, tag=f"fconvW{l}_{k}", pool=sg)
                nc.sync.dma_start(out=t2, in_=d["fconvW"][l, k * 128:(k + 1) * 128, :])
                convW[(l, k, "f")] = t2
        gateW = []
        for c in range(4):
            t = T([128, H], HDT, tag=f"gateW{c}", pool=sg)
            nc.sync.dma_start(out=t, in_=d["gateW"][c * 128:(c + 1) * 128, :])
            gateW.append(t)
        fvec = T([128, FV_N], tag="fvec_t", pool=sg)
        nc.sync.dma_start(out=fvec, in_=d["fvec"][:, :])

        degpc = T([128, gpc * 4], tag="degpc_t", pool=sg)
        nc.sync.dma_start(out=degpc, in_=d["degpc"][:, :])
        dinvpc = T([128, gpc * 4], tag="dinvpc", pool=sg)
        dtmp = T([128, gpc * 4], tag="dtmp", pool=sg)
        newton_rsqrt(degpc, dinvpc, dtmp, 3)

        ident = T([128, 128], tag="ident", pool=sg)
        make_identity(nc, ident)
        identb = T([128, 128], BF16, tag="identb", pool=sg)
        nc.vector.tensor_copy(out=identb, in_=ident)
        diagq = T([128, 128], tag="diagq", pool=sg)
        nc.scalar.mul(diagq, ident, 0.25)
        ones128f = T([128, 1], tag="ones128f", pool=sg)
        nc.vector.memset(ones128f, 1.0)
        ones128 = T([128, 1], HDT, tag="ones128", pool=sg)
        nc.scalar.copy(ones128, ones128f)
        ones1f = T([1, 128], tag="ones1f", pool=sg)
        nc.vector.memset(ones1f, 1.0)
        ones1 = T([1, 128], HDT, tag="ones1", pool=sg)
        nc.scalar.copy(ones1, ones1f)
        q4 = T([128, 4], BF16, tag="q4", pool=sg)
        nc.vector.memset(q4, 0.25)

        def fv(col, n=1):
            return fvec[:, col:col + n]

        def road(inT, Wk0, Wk1, Amat, b_col, nw_col, nb_col, nms_col, otag):
            # m = h @ W (node-major per source chunk)
            m = []
            for sc in range(4):
                ps = psM.tile([128, H], F32, name="psm", tag="psm", bufs=2)
                nc.tensor.matmul(ps, lhsT=inT[0][:, sc * 128:(sc + 1) * 128], rhs=Wk0, start=True, stop=False)
                nc.tensor.matmul(ps, lhsT=inT[1][:, sc * 128:(sc + 1) * 128], rhs=Wk1, start=False, stop=True)
                mt = T([128, H], HDT, tag=f"m_{sc}")
                nc.scalar.copy(mt, ps)
                m.append(mt)
            # out^T = A @ m  (feature-major), + bias
            cT = []
            for k in range(2):
                ps = psA.tile([128, NP], F32, name="psbig", tag="psbig", bufs=3)
                for sc in range(4):
                    nc.tensor.matmul(ps, lhsT=m[sc][:, k * 128:(k + 1) * 128], rhs=Amat[sc],
                                     start=(sc == 0), stop=(sc == 3))
                c = T([128, NP], tag=f"cT_{k}")
                nc.scalar.activation(out=c, in_=ps, func=AF.Identity, bias=fv(b_col + k))
                cT.append(c)
            # GraphNorm stats, both chunks batched into [128,2]
            s12 = T([128, 2], tag="s12")
            for k in range(2):
                nc.vector.reduce_sum(out=s12[:, k:k + 1], in_=cT[k][:, 0:NPG], axis=X)
            msn2 = T([128, 2], tag="msn2")
            nc.vector.scalar_tensor_tensor(out=msn2, in0=s12, scalar=-1.0 / NPG, in1=fv(nms_col, 2),
                                           op0=OP.mult, op1=OP.mult)
            ssq2 = T([128, 2], tag="ssq2")
            for k in range(2):
                junk = T([128, NPG], tag="junk")
                nc.scalar.activation(out=junk, in_=cT[k][:, 0:NPG], func=AF.Square, bias=msn2[:, k:k + 1],
                                     accum_out=ssq2[:, k:k + 1])
            u2 = T([128, 2], tag="u2")
            nc.vector.tensor_scalar(out=u2, in0=ssq2, scalar1=1.0 / NPG, scalar2=1e-5,
                                    op0=OP.mult, op1=OP.add)
            rstd2 = T([128, 2], tag="rstd2")
            ntmp2 = T([128, 2], tag="ntmp2")
            newton_rsqrt(u2, rstd2, ntmp2, 3)
            wr2 = T([128, 2], tag="wr2")
            nc.vector.tensor_tensor(out=wr2, in0=rstd2, in1=fv(nw_col, 2), op=OP.mult)
            bb2 = T([128, 2], tag="bb2")
            nc.vector.tensor_tensor(out=bb2, in0=wr2, in1=msn2, op=OP.mult)
            nc.vector.tensor_tensor(out=bb2, in0=bb2, in1=fv(nb_col, 2), op=OP.add)
            outT = []
            for k in range(2):
                oT = T([128, NP], HDT, tag=f"{otag}_{k}")
                nc.scalar.activation(out=oT, in_=cT[k], func=AF.Prelu, bias=bb2[:, k:k + 1],
                                     scale=wr2[:, k:k + 1], alpha=0.01)
                outT.append(oT)
            return outT

        for i in range(gpc):
            # ---- loads ----
            xT = T([128, NP], HDT, tag="xT_t")
            nc.sync.dma_start(out=xT, in_=d["xT"][i])
            eit = T([128, 4, W], mybir.dt.int16, tag="eit")
            evf = T([128, 4, W], tag="evf")
            for c in range(4):
                nc.sync.dma_start(out=eit[:, c, :], in_=d["ei"][i, c])
                nc.sync.dma_start(out=evf[:, c, :], in_=d["ev"][i, c])
            evb = T([128, 4, W], BF16, tag="evb")
            nc.vector.tensor_copy(out=evb, in_=evf)
            drow = T([1, NP], tag="drow")
            nc.sync.dma_start(out=drow, in_=d["degrow"][i:i + 1, :])

            # ---- dinv row (rsqrt of degree) broadcast to all partitions ----
            drow_n = T([1, NP], tag="drow_n")
            drow_t = T([1, NP], tag="drow_t")
            newton_rsqrt(drow, drow_n, drow_t, 2)
            drow_r = T([1, NP], HDT, tag="drow_r")
            nc.vector.tensor_copy(out=drow_r, in_=drow_n)
            psB = psA.tile([128, NP], F32, name="psbig", tag="psbig", bufs=3)
            nc.tensor.matmul(psB, lhsT=ones1[:, :], rhs=drow_r, start=True, stop=True)
            dinvB = T([128, NP], tag="dinvB")
            nc.vector.tensor_copy(out=dinvB, in_=psB)

            # ---- embedding ----
            hT = []
            for k in range(2):
                ps = psA.tile([128, NP], F32, name="psbig", tag="psbig", bufs=3)
                nc.tensor.matmul(ps, lhsT=embW[:, k * 128:(k + 1) * 128], rhs=xT, start=True, stop=True)
                t = T([128, NP], HDT, tag=f"hT_{k}")
                nc.scalar.activation(out=t, in_=ps, func=AF.Identity, bias=fv(FV_EMB_B + k))
                hT.append(t)

            # ---- main-road adjacency AT[s-chunk][s, d] ----
            AT = []
            for c in range(4):
                am = T([128, NP], BF16, tag="am")
                nc.gpsimd.local_scatter(out_ap=am[:, :], data_ap=evb[:, c, :], idxs_ap=eit[:, c, :],
                                        channels=128, num_elems=NP, num_idxs=W)
                amf = T([128, NP], tag="amf")
                nc.vector.tensor_copy(out=amf, in_=am)
                at = T([128, NP], HDT, tag=f"AT_{c}")
                nc.vector.scalar_tensor_tensor(out=at, in0=amf, scalar=dinvpc[:, i * 4 + c:i * 4 + c + 1],
                                               in1=dinvB, op0=OP.mult, op1=OP.mult)
                AT.append(at)

            # ---- knn road: cosine sim + top-k ----
            sq = []
            for k in range(2):
                t = T([128, NP], HDT, tag="sq")
                nc.scalar.square(t, f32(hT[k]))
                sq.append(t)
            psn = psN.tile([1, NP], F32, name="psn", tag="psn", bufs=1)
            nc.tensor.matmul(psn, lhsT=ones128[:, :], rhs=sq[0], start=True, stop=False)
            nc.tensor.matmul(psn, lhsT=ones128[:, :], rhs=sq[1], start=False, stop=True)
            nrm2 = T([1, NP], tag="nrm2")
            nc.vector.tensor_copy(out=nrm2, in_=psn)
            rin_n = T([1, NP], tag="rin_n")
            rin_t = T([1, NP], tag="rin_t")
            newton_rsqrt(nrm2, rin_n, rin_t, 2)
            rin_r = T([1, NP], HDT, tag="rin_r")
            nc.vector.tensor_copy(out=rin_r, in_=rin_n)
            psR = psA.tile([128, NP], F32, name="psbig", tag="psbig", bufs=3)
            nc.tensor.matmul(psR, lhsT=ones1[:, :], rhs=rin_r, start=True, stop=True)
            rb = T([128, NP], tag="rb")
            nc.vector.tensor_copy(out=rb, in_=psR)
            hnT = []
            for k in range(2):
                t = T([128, NP], HDT, tag=f"hnT_{k}")
                nc.vector.tensor_mul(t, f32(hT[k]), rb)
                hnT.append(t)

            afb = []
            for j in range(4):
                ps = psA.tile([128, NP], F32, name="psbig", tag="psbig", bufs=3)
                nc.tensor.matmul(ps, lhsT=hnT[0][:, j * 128:(j + 1) * 128], rhs=hnT[0], start=True, stop=False)
                nc.tensor.matmul(ps, lhsT=hnT[1][:, j * 128:(j + 1) * 128], rhs=hnT[1], start=False, stop=True)
                sim = T([128, NP], tag="sim")
                nc.scalar.copy(sim, ps)
                nc.vector.memset(sim[:, NPG:NP], -1e30)
                mx = T([128, 8], tag="mx")
                mi = T([128, 8], mybir.dt.uint16, tag="mi")
                nc.vector.max(mx, sim)
                nc.vector.max_index(mi, mx, sim)
                idx4 = T([128, 4], mybir.dt.int16, tag="idx4")
                nc.vector.memset(idx4, -1)
                rows = NPG - 384 if j == 3 else 128
                nc.vector.tensor_copy(out=idx4[0:rows, 0:3], in_=mi[0:rows, 0:3].bitcast(mybir.dt.int16))
                af = T([128, NP], BF16, tag=f"afb_{j}")
                nc.gpsimd.local_scatter(out_ap=af[:, :], data_ap=q4[:, :], idxs_ap=idx4[:, :],
                                        channels=128, num_elems=NP, num_idxs=4)
                afb.append(af)
            AfT = []
            for s in range(4):
                pst = psT.tile([128, NP], BF16, name="pst", tag="pst", bufs=2)
                for c in range(4):
                    nc.tensor.transpose(out=pst[:, c * 128:(c + 1) * 128], in_=afb[c][:, s * 128:(s + 1) * 128],
                                        identity=identb)
                t = T([128, NP], HDT, tag=f"AfT_{s}")
                nc.vector.tensor_copy(out=t, in_=pst)
                if s < 3:
                    nc.vector.tensor_add(t[:, s * 128:(s + 1) * 128], f32(t[:, s * 128:(s + 1) * 128]), diagq)
                else:
                    nc.vector.tensor_add(t[0:NPG - 384, 384:NPG], f32(t[0:NPG - 384, 384:NPG]),
                                         diagq[0:NPG - 384, 0:NPG - 384])
                AfT.append(t)

            # ---- layers ----
            prevT = hT
            curT = hT
            all0 = None
            for l in range(L):
                base = FV_L + l * 16
                h1 = road(curT, convW[(l, 0)], convW[(l, 1)], AT, base + 0, base + 2, base + 4, base + 6, "h1")
                h2 = road(h1, convW[(l, 0, "f")], convW[(l, 1, "f")], AfT, base + 8, base + 10, base + 12, base + 14, "h2")
                newT = []
                for k in range(2):
                    ps = psA.tile([128, NP], F32, name="psbig", tag="psbig", bufs=3)
                    for c in range(4):
                        rhs = h1[c] if c < 2 else h2[c - 2]
                        nc.tensor.matmul(ps, lhsT=gateW[c][:, k * 128:(k + 1) * 128], rhs=rhs,
                                         start=(c == 0), stop=(c == 3))
                    gT = T([128, NP], tag="gT")
                    nc.scalar.activation(out=gT, in_=ps, func=AF.Sigmoid, bias=fv(FV_GATE_B + k))
                    dT = T([128, NP], tag="dT")
                    nc.vector.tensor_sub(dT, f32(h1[k]), f32(h2[k]))
                    t2 = T([128, NP], tag="t2")
                    nc.vector.tensor_mul(t2, gT, dT)
                    nc.vector.tensor_add(t2, t2, f32(h2[k]))
                    hn = T([128, NP], HDT, tag=f"hn{l}_{k}")
                    nc.vector.tensor_add(hn, t2, f32(prevT[k]))
                    newT.append(hn)
                if l == 0:
                    all0 = newT
                prevT = newT
                curT = newT

            # ---- pooling: gf = (pool(all0) + 2*pool(all1)) / NPG ----
            gfo = T([128, 2], tag="gfo")
            for k in range(2):
                r0 = T([128, 1], tag="r0")
                nc.vector.reduce_sum(out=r0, in_=f32(all0[k])[:, 0:NPG], axis=X)
                r1 = T([128, 1], tag="r1")
                nc.vector.reduce_sum(out=r1, in_=f32(curT[k])[:, 0:NPG], axis=X)
                nc.vector.scalar_tensor_tensor(out=gfo[:, k:k + 1], in0=r1, scalar=2.0, in1=r0,
                                               op0=OP.mult, op1=OP.add)
            nc.vector.tensor_scalar_mul(gfo, gfo, 1.0 / NPG)
            nc.sync.dma_start(out=d["gf"][i].rearrange("(k p) -> p k", p=128), in_=gfo)


def prep_inputs(inputs):
    """Build the 8 per-core input maps from full-problem inputs."""
    x = np.asarray(inputs["x"], np.float32)
    edge_index = np.asarray(inputs["edge_index"], np.int64)
    batch = np.asarray(inputs["batch"], np.int64)
    N = G * NPG
    assert x.shape == (N, IN)
    assert np.array_equal(batch, np.repeat(np.arange(G), NPG)), "non-uniform batch unsupported"

    src, dst = edge_index[0], edge_index[1]
    gs = src // NPG
    assert np.array_equal(dst // NPG, gs), "cross-graph edges unsupported"
    sl = src % NPG
    dl = dst % NPG

    deg = np.bincount(dst, minlength=N).astype(np.float32) + 1.0

    # unique (g, s, d) with multiplicity, self loops appended
    gg = np.arange(G, dtype=np.int64).repeat(NPG)
    nn = np.tile(np.arange(NPG, dtype=np.int64), G)
    g_all = np.concatenate([gs, gg])
    s_all = np.concatenate([sl, nn])
    d_all = np.concatenate([dl, nn])
    key = (g_all * NPG + s_all) * NPG + d_all
    uk, cnt = np.unique(key, return_counts=True)
    ud = (uk % NPG).astype(np.int16)
    row = (uk // NPG).astype(np.int64)  # g*NPG + s
    row_start = np.searchsorted(row, np.arange(N))
    pos = np.arange(len(row)) - row_start[row]
    assert pos.max() < W, f"out-degree overflow: {pos.max() + 1} > {W}"
    EI = np.full((N, W), -1, np.int16)
    EV = np.zeros((N, W), np.float32)
    EI[row, pos] = ud
    EV[row, pos] = cnt

    rng = np.random.default_rng(12345)
    wts = dict(
        embW=np.ascontiguousarray(np.asarray(inputs["emb_W"], np.float32)),
        convW=np.ascontiguousarray(np.asarray(inputs["conv_W"], np.float32)[:L]),
        fconvW=np.ascontiguousarray(np.asarray(inputs["fconv_W"], np.float32)[:L]),
        gateW=np.ascontiguousarray(np.asarray(inputs["gate_W"], np.float32)),
    )
    fvec = np.zeros((128, FV_N), np.float32)

    def setv(col, vec):
        fvec[:, col] = vec[0:128]
        fvec[:, col + 1] = vec[128:256]

    setv(FV_EMB_B, np.asarray(inputs["emb_b"], np.float32))
    setv(FV_GATE_B, np.asarray(inputs["gate_b"], np.float32))
    for l in range(L):
        base = FV_L + l * 16
        setv(base + 0, np.asarray(inputs["conv_b"], np.float32)[l])
        setv(base + 2, np.asarray(inputs["norm_w"], np.float32)[l])
        setv(base + 4, np.asarray(inputs["norm_b"], np.float32)[l])
        setv(base + 6, np.asarray(inputs["norm_ms"], np.float32)[l])
        setv(base + 8, np.asarray(inputs["fconv_b"], np.float32)[l])
        setv(base + 10, np.asarray(inputs["fnorm_w"], np.float32)[l])
        setv(base + 12, np.asarray(inputs["fnorm_b"], np.float32)[l])
        setv(base + 14, np.asarray(inputs["fnorm_ms"], np.float32)[l])

    in_maps = []
    for c in range(N_CORES):
        g0, ng = STARTS[c], NGS[c]
        xT = np.zeros((GPC, IN, NP), np.float32)
        ei_c = np.full((GPC, 4, 128, W), -1, np.int16)
        ev_c = np.zeros((GPC, 4, 128, W), np.float32)
        degpc = np.ones((GPC, 4, 128), np.float32)
        degrow = np.ones((GPC, NP), np.float32)
        for j in range(GPC):
            if j < ng:
                g = g0 + j
                xg = x[g * NPG:(g + 1) * NPG]
            else:
                xg = rng.standard_normal((NPG, IN)).astype(np.float32)
            xT[j, :, 0:NPG] = xg.T
            if j < ng:
                eig = np.full((NP, W), -1, np.int16)
                evg = np.zeros((NP, W), np.float32)
                eig[0:NPG] = EI[g * NPG:(g + 1) * NPG]
                evg[0:NPG] = EV[g * NPG:(g + 1) * NPG]
                ei_c[j] = eig.reshape(4, 128, W)
                ev_c[j] = evg.reshape(4, 128, W)
                dg = np.ones(NP, np.float32)
                dg[0:NPG] = deg[g * NPG:(g + 1) * NPG]
                degpc[j] = dg.reshape(4, 128)
                degrow[j] = dg
        in_maps.append(dict(
            xT=xT,
            ei=ei_c,
            ev=ev_c,
            degpc=np.ascontiguousarray(degpc.reshape(GPC * 4, 128).T),
            degrow=degrow,
            fvec=fvec,
            **wts,
        ))
    return in_maps


_prog_cache = {}


def _get_program():
    if "nc" not in _prog_cache:
        _prog_cache["nc"] = build_program(GPC)
    return _prog_cache["nc"]


def kernel(**inputs):
    in_maps = prep_inputs(inputs)
    nc = _get_program()
    trace = os.environ.get("KERNEL_TRACE", "0") == "1"
    kw = {}
    if trace:
        import antenv
        p = "/opt/trn_rl_repo/antenv"
        if p not in antenv.__path__:
            antenv.__path__.append(p)
        from antenv.axon_hooks import get_axon_ntff_profile_hook, set_axon_ntff_profile_hook
        if get_axon_ntff_profile_hook() is None:
            from trn_agent_boot.trn_boot import _ntff_profile_via_ctypes
            set_axon_ntff_profile_hook(_ntff_profile_via_ctypes("/opt/axon/libaxon_pjrt.so"))
        from concourse import bass_utils as _bu
        _bu.upload_artifacts = lambda tmpdir: "local://" + tmpdir
        base = os.environ.get("KERNEL_TRACE_DIR")
        if base:
            _prog_cache["run_id"] = _prog_cache.get("run_id", 0) + 1
            tdir = os.path.join(base, f"run{_prog_cache['run_id']}")
            os.makedirs(tdir, exist_ok=True)
        else:
            tdir = None
        kw = dict(trace=True, tmpdir=tdir)
    res = run_bass_kernel_spmd(nc, in_maps, core_ids=list(range(N_CORES)), **kw)
    if trace:
        print(f"HW exec time: {res.exec_time_ns} ns")
    out = np.zeros((G, H), np.float32)
    for c in range(N_CORES):
        g0, ng = STARTS[c], NGS[c]
        out[g0:g0 + ng] = res.results[c]["gf"][0:ng]
    return out


# revision 10
# speedup vs baseline: 1.3940x; 1.0628x over previous
"""DualRoadGNN Trainium2 kernel: 8-core SPMD, sharded by graph.

Layout: feature-major per graph ([H partitions, node columns]); graphs padded
500 -> 512 nodes. GCN message passing runs as dense matmuls against per-graph
adjacency matrices built on device from host-shipped integer edge lists
(local_scatter of multiplicities, scaled by rsqrt-degree outer products).
KNN road: cosine sim via PE matmul, top-k via DVE max/max_index, adjacency
via local_scatter + PE transpose.
"""
import contextlib
import os
import sys

sys.path.insert(0, "/opt/trn_rl_repo")
import numpy as np

import concourse.bacc as bacc
import concourse.tile as tile
from concourse import mybir
from concourse.bass_utils import run_bass_kernel_spmd
from concourse.masks import make_identity

G, NPG, NP = 100, 500, 512
IN, H, L = 128, 256, 2   # L = executed layer iterations (range(3-1) in the model)
W = 64                   # max unique out-edges per source node (incl self loop)
N_CORES = 8
GPC = 13                 # graph slots per core
STARTS = [0, 13, 26, 39, 52, 64, 76, 88, 100]
NGS = [STARTS[i + 1] - STARTS[i] for i in range(N_CORES)]
F32 = mybir.dt.float32
BF16 = mybir.dt.bfloat16
MM_MODE = os.environ.get("KERNEL_MMDT", "bf16")
HDT = {"f32": F32, "f32r": mybir.dt.float32r, "bf16": BF16}[MM_MODE]
KDT = F32 if MM_MODE == "f32" else mybir.dt.float32r

# fvec column map
FV_EMB_B = 0
FV_GATE_B = 2
FV_L = 4   # then per layer: conv_b, norm_w, norm_b, norm_ms, fconv_b, fnorm_w, fnorm_b, fnorm_ms
FV_N = 4 + L * 16


def build_program(gpc):
    nc = bacc.Bacc("TRN2", target_bir_lowering=False, debug=False, num_devices=N_CORES)
    d = {}
    d["xT"] = nc.dram_tensor("xT", [gpc, IN, NP], HDT, kind="ExternalInput")
    d["ei"] = nc.dram_tensor("ei", [gpc, 4, 128, W], mybir.dt.int16, kind="ExternalInput")
    d["ev"] = nc.dram_tensor("ev", [gpc, 4, 128, W], BF16, kind="ExternalInput")
    d["degpc"] = nc.dram_tensor("degpc", [128, gpc * 4], F32, kind="ExternalInput")
    d["degrow"] = nc.dram_tensor("degrow", [gpc, NP], F32, kind="ExternalInput")
    d["embW"] = nc.dram_tensor("embW", [IN, H], HDT, kind="ExternalInput")
    d["convW"] = nc.dram_tensor("convW", [L, H, H], HDT, kind="ExternalInput")
    d["fconvW"] = nc.dram_tensor("fconvW", [L, H, H], HDT, kind="ExternalInput")
    d["gateW"] = nc.dram_tensor("gateW", [2 * H, H], HDT, kind="ExternalInput")
    d["fvec"] = nc.dram_tensor("fvec", [128, FV_N], F32, kind="ExternalInput")
    d["gf"] = nc.dram_tensor("gf", [gpc, H], F32, kind="ExternalOutput")

    with tile.TileContext(nc) as tc:
        _emit(nc, tc, gpc, d)
    nc.compile()
    return nc


def _emit(nc, tc, gpc, d):
    AF = mybir.ActivationFunctionType
    OP = mybir.AluOpType
    X = mybir.AxisListType.X
    I32 = mybir.dt.int32

    ctx = contextlib.ExitStack()
    with ctx:
        sg = ctx.enter_context(tc.tile_pool(name="singles", bufs=1))
        pg = ctx.enter_context(tc.tile_pool(name="pg", bufs=3))
        psA = ctx.enter_context(tc.tile_pool(name="psA", bufs=4, space="PSUM"))
        psM = ctx.enter_context(tc.tile_pool(name="psM", bufs=2, space="PSUM"))
        psT = ctx.enter_context(tc.tile_pool(name="psT", bufs=2, space="PSUM"))

        def T(shape, dtype=F32, tag=None, pool=pg, bufs=None):
            kw = {} if bufs is None else {"bufs": bufs}
            return pool.tile(shape, dtype, name=tag, tag=tag, **kw)

        def f32(ap):
            # read-view for non-matmul engines; f32r needs a bitcast, bf16 is native
            return ap.bitcast(F32) if ap.dtype == mybir.dt.float32r else ap

        def newton_rsqrt(v_ap, out_tile, tmp_tile, iters):
            """out = 1/sqrt(v) via bit-trick + Newton (DVE only, no ACT tables)."""
            y = out_tile
            nc.vector.tensor_scalar(out=y.bitcast(I32), in0=v_ap.bitcast(I32), scalar1=1, scalar2=None,
                                    op0=OP.arith_shift_right)
            nc.vector.tensor_scalar(out=y.bitcast(I32), in0=y.bitcast(I32), scalar1=-1, scalar2=0x5F3759DF,
                                    op0=OP.mult, op1=OP.add)
            for _ in range(iters):
                nc.vector.tensor_mul(tmp_tile, y, y)
                nc.vector.tensor_mul(tmp_tile, tmp_tile, v_ap)
                nc.vector.tensor_scalar(out=tmp_tile, in0=tmp_tile, scalar1=-0.5, scalar2=1.5,
                                        op0=OP.mult, op1=OP.add)
                nc.vector.tensor_mul(y, y, tmp_tile)
            return y

        # --- resident constants/weights ---
        embW = T([128, H], HDT, tag="embW_t", pool=sg)
        nc.sync.dma_start(out=embW, in_=d["embW"][:, :])
        convW = {}
        for l in range(L):
            for k in range(2):
                t = T([128, H], HDT, tag=f"convW{l}_{k}", pool=sg)
                nc.sync.dma_start(out=t, in_=d["convW"][l, k * 128:(k + 1) * 128, :])
                convW[(l, k)] = t
                t2 = T([128, H], HDT, tag=f"fconvW{l}_{k}", pool=sg)
                nc.sync.dma_start(out=t2, in_=d["fconvW"][l, k * 128:(k + 1) * 128, :])
                convW[(l, k, "f")] = t2
        gateW = []
        for c in range(4):
            t = T([128, H], HDT, tag=f"gateW{c}", pool=sg)
            nc.sync.dma_start(out=t, in_=d["gateW"][c * 128:(c + 1) * 128, :])
            gateW.append(t)
        fvec = T([128, FV_N], tag="fvec_t", pool=sg)
        nc.sync.dma_start(out=fvec, in_=d["fvec"][:, :])

        degpc = T([128, gpc * 4], tag="degpc_t", pool=sg)
        nc.sync.dma_start(out=degpc, in_=d["degpc"][:, :])
        dinvpc = T([128, gpc * 4], tag="dinvpc", pool=sg)
        dtmp = T([128, gpc * 4], tag="dtmp", pool=sg)
        newton_rsqrt(degpc, dinvpc, dtmp, 3)

        ident = T([128, 128], tag="ident", pool=sg)
        make_identity(nc, ident)
        identb = T([128, 128], BF16, tag="identb", pool=sg)
        nc.vector.tensor_copy(out=identb, in_=ident)
        diagq = T([128, 128], HDT, tag="diagq", pool=sg)
        nc.scalar.mul(diagq, ident, 0.25)
        onesf = T([128, 1], tag="onesf", pool=sg)
        nc.vector.memset(onesf, 1.0)
        ones128 = T([128, 1], KDT, tag="ones128", pool=sg)
        nc.scalar.copy(ones128, onesf)
        ones1f = T([1, 128], tag="ones1f", pool=sg)
        nc.vector.memset(ones1f, 1.0)
        ones1 = T([1, 128], HDT, tag="ones1", pool=sg)
        nc.scalar.copy(ones1, ones1f)
        ones1r = ones1
        if KDT != HDT:
            ones1r = T([1, 128], KDT, tag="ones1r", pool=sg)
            nc.scalar.copy(ones1r, ones1f)
        q4 = T([128, 4], BF16, tag="q4", pool=sg)
        nc.vector.memset(q4, 0.25)

        def fv(col, n=1):
            return fvec[:, col:col + n]

        def road(inT, Wk0, Wk1, Amat, b_col, nw_col, nb_col, nms_col, otag):
            # m = h @ W (node-major per source chunk)
            m = []
            for sc in range(4):
                ps = psM.tile([128, H], F32, name="psm", tag="psm", bufs=2)
                nc.tensor.matmul(ps, lhsT=inT[0][:, sc * 128:(sc + 1) * 128], rhs=Wk0, start=True, stop=False)
                nc.tensor.matmul(ps, lhsT=inT[1][:, sc * 128:(sc + 1) * 128], rhs=Wk1, start=False, stop=True)
                mt = T([128, H], HDT, tag=f"m_{sc}")
                nc.scalar.copy(mt, ps)
                m.append(mt)
            # out^T = A @ m  (feature-major), + conv bias
            cT = []
            for k in range(2):
                ps = psA.tile([128, NP], F32, name="psbig", tag="psbig", bufs=4)
                for sc in range(4):
                    nc.tensor.matmul(ps, lhsT=m[sc][:, k * 128:(k + 1) * 128], rhs=Amat[sc],
                                     start=(sc == 0), stop=(sc == 3))
                c = T([128, NP], tag=f"cT_{k}", bufs=2)
                nc.scalar.activation(out=c, in_=ps, func=AF.Identity, bias=fv(b_col + k))
                cT.append(c)
            # GraphNorm stats, both chunks batched into [128,2]
            s12 = T([128, 2], tag="s12")
            for k in range(2):
                nc.vector.reduce_sum(out=s12[:, k:k + 1], in_=cT[k][:, 0:NPG], axis=X)
            msn2 = T([128, 2], tag="msn2")
            nc.vector.scalar_tensor_tensor(out=msn2, in0=s12, scalar=-1.0 / NPG, in1=fv(nms_col, 2),
                                           op0=OP.mult, op1=OP.mult)
            ssq2 = T([128, 2], tag="ssq2")
            for k in range(2):
                junk = T([128, NPG], tag="junk", bufs=2)
                nc.scalar.activation(out=junk, in_=cT[k][:, 0:NPG], func=AF.Square, bias=msn2[:, k:k + 1],
                                     accum_out=ssq2[:, k:k + 1])
            u2 = T([128, 2], tag="u2")
            nc.vector.tensor_scalar(out=u2, in0=ssq2, scalar1=1.0 / NPG, scalar2=1e-5,
                                    op0=OP.mult, op1=OP.add)
            rstd2 = T([128, 2], tag="rstd2")
            ntmp2 = T([128, 2], tag="ntmp2")
            newton_rsqrt(u2, rstd2, ntmp2, 3)
            wr2 = T([128, 2], tag="wr2")
            nc.vector.tensor_tensor(out=wr2, in0=rstd2, in1=fv(nw_col, 2), op=OP.mult)
            bb2 = T([128, 2], tag="bb2")
            nc.vector.tensor_tensor(out=bb2, in0=wr2, in1=msn2, op=OP.mult)
            nc.vector.tensor_tensor(out=bb2, in0=bb2, in1=fv(nb_col, 2), op=OP.add)
            outT = []
            for k in range(2):
                oT = T([128, NP], HDT, tag=f"{otag}_{k}")
                nc.scalar.activation(out=oT, in_=cT[k], func=AF.Prelu, bias=bb2[:, k:k + 1],
                                     scale=wr2[:, k:k + 1], alpha=0.01)
                outT.append(oT)
            return outT

        for i in range(gpc):
            # ---- loads ----
            xT = T([128, NP], HDT, tag="xT_t")
            nc.sync.dma_start(out=xT, in_=d["xT"][i])
            eit = T([128, 4, W], mybir.dt.int16, tag="eit")
            evb = T([128, 4, W], BF16, tag="evb")
            for c in range(4):
                nc.sync.dma_start(out=eit[:, c, :], in_=d["ei"][i, c])
                nc.sync.dma_start(out=evb[:, c, :], in_=d["ev"][i, c])
            drow = T([1, NP], tag="drow")
            nc.sync.dma_start(out=drow, in_=d["degrow"][i:i + 1, :])

            # ---- dinv row (rsqrt degree), broadcast to all partitions ----
            drow_n = T([1, NP], tag="drow_n")
            drow_t = T([1, NP], tag="drow_t")
            newton_rsqrt(drow, drow_n, drow_t, 2)
            drow_r = T([1, NP], HDT, tag="drow_r")
            nc.vector.tensor_copy(out=drow_r, in_=drow_n)
            psB = psA.tile([128, NP], F32, name="psbig", tag="psbig", bufs=4)
            nc.tensor.matmul(psB, lhsT=ones1[:, :], rhs=drow_r, start=True, stop=True)
            dinvB = T([128, NP], tag="dinvB", bufs=2)
            nc.vector.tensor_copy(out=dinvB, in_=psB)

            # ---- embedding (hTr: knn-fidelity copy; hT: matmul dtype) ----
            hT = []
            hTr = []
            for k in range(2):
                ps = psA.tile([128, NP], F32, name="psbig", tag="psbig", bufs=4)
                nc.tensor.matmul(ps, lhsT=embW[:, k * 128:(k + 1) * 128], rhs=xT, start=True, stop=True)
                if KDT != HDT:
                    tr = T([128, NP], KDT, tag=f"hTr_{k}")
                    nc.scalar.activation(out=tr, in_=ps, func=AF.Identity, bias=fv(FV_EMB_B + k))
                    t = T([128, NP], HDT, tag=f"hT_{k}")
                    nc.vector.tensor_copy(out=t, in_=f32(tr))
                else:
                    t = T([128, NP], HDT, tag=f"hT_{k}")
                    nc.scalar.activation(out=t, in_=ps, func=AF.Identity, bias=fv(FV_EMB_B + k))
                    tr = t
                hT.append(t)
                hTr.append(tr)

            # ---- main-road adjacency AT[s-chunk][s, d] ----
            AT = []
            for c in range(4):
                am = T([128, NP], BF16, tag="am")
                nc.gpsimd.local_scatter(out_ap=am[:, :], data_ap=evb[:, c, :], idxs_ap=eit[:, c, :],
                                        channels=128, num_elems=NP, num_idxs=W)
                amf = T([128, NP], tag="amf", bufs=2)
                nc.vector.tensor_copy(out=amf, in_=am)
                at = T([128, NP], HDT, tag=f"AT_{c}")
                nc.vector.scalar_tensor_tensor(out=at, in0=amf, scalar=dinvpc[:, i * 4 + c:i * 4 + c + 1],
                                               in1=dinvB, op0=OP.mult, op1=OP.mult)
                AT.append(at)

            # ---- knn road: cosine sim + top-k ----
            sq = []
            for k in range(2):
                t = T([128, NP], KDT, tag="sq")
                nc.scalar.square(t, f32(hTr[k]))
                sq.append(t)
            psn = psA.tile([1, NP], F32, name="psbig", tag="psbig", bufs=4)
            nc.tensor.matmul(psn, lhsT=ones128[:, :], rhs=sq[0], start=True, stop=False)
            nc.tensor.matmul(psn, lhsT=ones128[:, :], rhs=sq[1], start=False, stop=True)
            nrm2 = T([1, NP], tag="nrm2")
            nc.vector.tensor_copy(out=nrm2, in_=psn)
            rin_n = T([1, NP], tag="rin_n")
            rin_t = T([1, NP], tag="rin_t")
            newton_rsqrt(nrm2, rin_n, rin_t, 2)
            rin_r = T([1, NP], KDT, tag="rin_r")
            nc.vector.tensor_copy(out=rin_r, in_=rin_n)
            psR = psA.tile([128, NP], F32, name="psbig", tag="psbig", bufs=4)
            nc.tensor.matmul(psR, lhsT=ones1r[:, :], rhs=rin_r, start=True, stop=True)
            rb = T([128, NP], tag="rb", bufs=2)
            nc.vector.tensor_copy(out=rb, in_=psR)
            hnT = []
            for k in range(2):
                t = T([128, NP], KDT, tag=f"hnT_{k}")
                nc.vector.tensor_mul(t, f32(hTr[k]), rb)
                hnT.append(t)

            afb = []
            for j in range(4):
                ps = psA.tile([128, NP], F32, name="psbig", tag="psbig", bufs=4)
                nc.tensor.matmul(ps, lhsT=hnT[0][:, j * 128:(j + 1) * 128], rhs=hnT[0], start=True, stop=False)
                nc.tensor.matmul(ps, lhsT=hnT[1][:, j * 128:(j + 1) * 128], rhs=hnT[1], start=False, stop=True)
                sim = T([128, NP], tag="sim", bufs=2)
                nc.scalar.copy(sim, ps)
                nc.vector.memset(sim[:, NPG:NP], -1e30)
                mx = T([128, 8], tag="mx")
                mi = T([128, 8], mybir.dt.uint16, tag="mi")
                nc.vector.max(mx, sim)
                nc.vector.max_index(mi, mx, sim)
                idx4 = T([128, 4], mybir.dt.int16, tag="idx4")
                nc.vector.memset(idx4, -1)
                rows = NPG - 384 if j == 3 else 128
                nc.vector.tensor_copy(out=idx4[0:rows, 0:3], in_=mi[0:rows, 0:3].bitcast(mybir.dt.int16))
                af = T([128, NP], BF16, tag=f"afb_{j}")
                nc.gpsimd.local_scatter(out_ap=af[:, :], data_ap=q4[:, :], idxs_ap=idx4[:, :],
                                        channels=128, num_elems=NP, num_idxs=4)
                afb.append(af)
            AfT = []
            for s in range(4):
                pst = psT.tile([128, NP], BF16, name="pst", tag="pst", bufs=2)
                for c in range(4):
                    nc.tensor.transpose(out=pst[:, c * 128:(c + 1) * 128], in_=afb[c][:, s * 128:(s + 1) * 128],
                                        identity=identb)
                t = T([128, NP], HDT, tag=f"AfT_{s}")
                nc.vector.tensor_copy(out=t, in_=pst)
                if s < 3:
                    nc.vector.tensor_add(t[:, s * 128:(s + 1) * 128], f32(t[:, s * 128:(s + 1) * 128]), diagq)
                else:
                    nc.vector.tensor_add(t[0:NPG - 384, 384:NPG], f32(t[0:NPG - 384, 384:NPG]),
                                         diagq[0:NPG - 384, 0:NPG - 384])
                AfT.append(t)

            # ---- layers ----
            prevT = hT
            curT = hT
            all0 = None
            for l in range(L):
                base = FV_L + l * 16
                h1 = road(curT, convW[(l, 0)], convW[(l, 1)], AT, base + 0, base + 2, base + 4, base + 6, "h1")
                h2 = road(h1, convW[(l, 0, "f")], convW[(l, 1, "f")], AfT, base + 8, base + 10, base + 12, base + 14, "h2")
                newT = []
                for k in range(2):
                    ps = psA.tile([128, NP], F32, name="psbig", tag="psbig", bufs=4)
                    for c in range(4):
                        rhs = h1[c] if c < 2 else h2[c - 2]
                        nc.tensor.matmul(ps, lhsT=gateW[c][:, k * 128:(k + 1) * 128], rhs=rhs,
                                         start=(c == 0), stop=(c == 3))
                    gT = T([128, NP], HDT, tag="gT", bufs=2)
                    nc.scalar.activation(out=gT, in_=ps, func=AF.Sigmoid, bias=fv(FV_GATE_B + k))
                    dT = T([128, NP], HDT, tag="dT", bufs=2)
                    nc.vector.tensor_sub(dT, f32(h1[k]), f32(h2[k]))
                    t2 = T([128, NP], HDT, tag="t2", bufs=2)
                    nc.vector.tensor_mul(t2, f32(gT), f32(dT))
                    nc.vector.tensor_add(t2, f32(t2), f32(h2[k]))
                    hn = T([128, NP], HDT, tag=f"hn{l}_{k}")
                    nc.vector.tensor_add(hn, f32(t2), f32(prevT[k]))
                    newT.append(hn)
                if l == 0:
                    all0 = newT
                prevT = newT
                curT = newT

            # ---- pooling: gf = (pool(all0) + 2*pool(all1)) / NPG ----
            gfo = T([128, 2], tag="gfo")
            for k in range(2):
                r0 = T([128, 1], tag="r0")
                nc.vector.reduce_sum(out=r0, in_=f32(all0[k])[:, 0:NPG], axis=X)
                r1 = T([128, 1], tag="r1")
                nc.vector.reduce_sum(out=r1, in_=f32(curT[k])[:, 0:NPG], axis=X)
                nc.vector.scalar_tensor_tensor(out=gfo[:, k:k + 1], in0=r1, scalar=2.0, in1=r0,
                                               op0=OP.mult, op1=OP.add)
            nc.vector.tensor_scalar_mul(gfo, gfo, 1.0 / NPG)
            nc.sync.dma_start(out=d["gf"][i].rearrange("(k p) -> p k", p=128), in_=gfo)


def prep_inputs(inputs):
    """Build the 8 per-core input maps from full-problem inputs."""
    x = np.asarray(inputs["x"], np.float32)
    edge_index = np.asarray(inputs["edge_index"], np.int64)
    batch = np.asarray(inputs["batch"], np.int64)
    N = G * NPG
    assert x.shape == (N, IN)
    assert np.array_equal(batch, np.repeat(np.arange(G), NPG)), "non-uniform batch unsupported"

    src, dst = edge_index[0], edge_index[1]
    gs = src // NPG
    assert np.array_equal(dst // NPG, gs), "cross-graph edges unsupported"
    sl = src % NPG
    dl = dst % NPG

    deg = np.bincount(dst, minlength=N).astype(np.float32) + 1.0

    # unique (g, s, d) with multiplicity, self loops appended
    gg = np.arange(G, dtype=np.int64).repeat(NPG)
    nn = np.tile(np.arange(NPG, dtype=np.int64), G)
    g_all = np.concatenate([gs, gg])
    s_all = np.concatenate([sl, nn])
    d_all = np.concatenate([dl, nn])
    key = (g_all * NPG + s_all) * NPG + d_all
    uk, cnt = np.unique(key, return_counts=True)
    ud = (uk % NPG).astype(np.int16)
    row = (uk // NPG).astype(np.int64)  # g*NPG + s
    row_start = np.searchsorted(row, np.arange(N))
    pos = np.arange(len(row)) - row_start[row]
    assert pos.max() < W, f"out-degree overflow: {pos.max() + 1} > {W}"
    EI = np.full((N, W), -1, np.int16)
    EV = np.zeros((N, W), np.float32)
    EI[row, pos] = ud
    EV[row, pos] = cnt

    import ml_dtypes
    global _bf
    _bf = ml_dtypes.bfloat16
    _wdt = _bf if HDT is BF16 else np.float32
    rng = np.random.default_rng(12345)
    wts = dict(
        embW=np.ascontiguousarray(np.asarray(inputs["emb_W"], np.float32)).astype(_wdt),
        convW=np.ascontiguousarray(np.asarray(inputs["conv_W"], np.float32)[:L]).astype(_wdt),
        fconvW=np.ascontiguousarray(np.asarray(inputs["fconv_W"], np.float32)[:L]).astype(_wdt),
        gateW=np.ascontiguousarray(np.asarray(inputs["gate_W"], np.float32)).astype(_wdt),
    )
    fvec = np.zeros((128, FV_N), np.float32)

    def setv(col, vec):
        fvec[:, col] = vec[0:128]
        fvec[:, col + 1] = vec[128:256]

    setv(FV_EMB_B, np.asarray(inputs["emb_b"], np.float32))
    setv(FV_GATE_B, np.asarray(inputs["gate_b"], np.float32))
    for l in range(L):
        base = FV_L + l * 16
        setv(base + 0, np.asarray(inputs["conv_b"], np.float32)[l])
        setv(base + 2, np.asarray(inputs["norm_w"], np.float32)[l])
        setv(base + 4, np.asarray(inputs["norm_b"], np.float32)[l])
        setv(base + 6, np.asarray(inputs["norm_ms"], np.float32)[l])
        setv(base + 8, np.asarray(inputs["fconv_b"], np.float32)[l])
        setv(base + 10, np.asarray(inputs["fnorm_w"], np.float32)[l])
        setv(base + 12, np.asarray(inputs["fnorm_b"], np.float32)[l])
        setv(base + 14, np.asarray(inputs["fnorm_ms"], np.float32)[l])

    in_maps = []
    for c in range(N_CORES):
        g0, ng = STARTS[c], NGS[c]
        xT = np.zeros((GPC, IN, NP), np.float32)
        ei_c = np.full((GPC, 4, 128, W), -1, np.int16)
        ev_c = np.zeros((GPC, 4, 128, W), np.float32)
        degpc = np.ones((GPC, 4, 128), np.float32)
        degrow = np.ones((GPC, NP), np.float32)
        for j in range(GPC):
            if j < ng:
                g = g0 + j
                xg = x[g * NPG:(g + 1) * NPG]
            else:
                xg = rng.standard_normal((NPG, IN)).astype(np.float32)
            xT[j, :, 0:NPG] = xg.T
            if j < ng:
                eig = np.full((NP, W), -1, np.int16)
                evg = np.zeros((NP, W), np.float32)
                eig[0:NPG] = EI[g * NPG:(g + 1) * NPG]
                evg[0:NPG] = EV[g * NPG:(g + 1) * NPG]
                ei_c[j] = eig.reshape(4, 128, W)
                ev_c[j] = evg.reshape(4, 128, W)
                dg = np.ones(NP, np.float32)
                dg[0:NPG] = deg[g * NPG:(g + 1) * NPG]
                degpc[j] = dg.reshape(4, 128)
                degrow[j] = dg
        in_maps.append(dict(
            xT=xT if HDT is not BF16 else xT.astype(_bf),
            ei=ei_c,
            ev=ev_c.astype(_bf),
            degpc=np.ascontiguousarray(degpc.reshape(GPC * 4, 128).T),
            degrow=degrow,
            fvec=fvec,
            **wts,
        ))
    return in_maps


_prog_cache = {}


def _get_program():
    if "nc" not in _prog_cache:
        _prog_cache["nc"] = build_program(GPC)
    return _prog_cache["nc"]


def kernel(**inputs):
    in_maps = prep_inputs(inputs)
    nc = _get_program()
    trace = os.environ.get("KERNEL_TRACE", "0") == "1"
    kw = {}
    if trace:
        import antenv
        p = "/opt/trn_rl_repo/antenv"
        if p not in antenv.__path__:
            antenv.__path__.append(p)
        from antenv.axon_hooks import get_axon_ntff_profile_hook, set_axon_ntff_profile_hook
        if get_axon_ntff_profile_hook() is None:
            from trn_agent_boot.trn_boot import _ntff_profile_via_ctypes
            set_axon_ntff_profile_hook(_ntff_profile_via_ctypes("/opt/axon/libaxon_pjrt.so"))
        from concourse import bass_utils as _bu
        _bu.upload_artifacts = lambda tmpdir: "local://" + tmpdir
        base = os.environ.get("KERNEL_TRACE_DIR")
        if base:
            _prog_cache["run_id"] = _prog_cache.get("run_id", 0) + 1
            tdir = os.path.join(base, f"run{_prog_cache['run_id']}")
            os.makedirs(tdir, exist_ok=True)
        else:
            tdir = None
        kw = dict(trace=True, tmpdir=tdir)
    res = run_bass_kernel_spmd(nc, in_maps, core_ids=list(range(N_CORES)), **kw)
    if trace:
        print(f"HW exec time: {res.exec_time_ns} ns")
    out = np.zeros((G, H), np.float32)
    for c in range(N_CORES):
        g0, ng = STARTS[c], NGS[c]
        out[g0:g0 + ng] = res.results[c]["gf"][0:ng]
    return out
